# revision 2
# baseline (speedup 1.0000x reference)
"""Trainium2 Bass kernel for nn_MinMaxQuantizer (per-channel symmetric log_2 quantizer).

Math (per row c of x[C, D], half = 2**(n_bits-1)):
    maxe    = round(log2(max|x[c, :]|))               # exponent of row max
    z       = maxe - (half - 1)                       # min kept exponent
    e       = round(log2(|x|))                        # per element
    out     = sign(x) * 2^e   if e >= z else 0

Implementation trick: y = x * c with c = (1 - 2^-9)/sqrt(2), converted to
bf16, places floor_exp(y) = e - 1 exactly (the bf16 round-to-nearest carry
at the exponent boundary implements the "mantissa >= sqrt(2)" test).  Then

    p   = y & 0xFF80          (u16 view: sign + exponent of y, bf16 +-2^(e-1))
    q   = int16(p * 2^(1-z))  = +-2^(e-z), converts to 0 when e < z (|v|<1)
    out = fp8e4m3(q * 2^z)    = +-2^e exactly (power of two, e in [-5, 3])

Row max of |p| is 2^(maxe-1) exactly, so all per-row parameters are pure
exponent-bit arithmetic (multiples of 2^7/2^23 stay exact in the fp32-internal
ALUs).  fp8 output halves+halves HBM write traffic; the host expands to f32
losslessly (all representable values are powers of two or zero).

Sharding: rows 4096 -> 8 cores x 512 rows, zero communication.
"""

import sys

import numpy as np

_REPO = "/opt/trn_rl_repo"

N_ROWS = 4096
N_COLS = 11008
N_CORES = 8
ROWS_PER_CORE = N_ROWS // N_CORES  # 512
P = 128
N_SLAB = ROWS_PER_CORE // P  # 4
N_CH = 4
W = N_COLS // N_CH  # 2752

# y = x * C_ADJ: bf16 rounds y up across the 2^k boundary exactly when
# mantissa(x) >= sqrt(2)  (boundary of round(log2)).
C_ADJ = float(np.float32((1.0 - 2.0 ** -9) / np.sqrt(2.0)))
_EXPSIGN16 = 0xFF80  # u16 mask: sign + exponent of bf16


def _ensure_path():
    if _REPO not in sys.path:
        sys.path.insert(0, _REPO)


def _build(n_bits: int):
    _ensure_path()
    import concourse.bacc as bacc
    import concourse.mybir as mybir
    import concourse.tile as tile

    dt = mybir.dt
    Alu = mybir.AluOpType
    Act = mybir.ActivationFunctionType
    X = mybir.AxisListType.X

    half = 2 ** (n_bits - 1)
    # bits16(2^z) = bits16(rowmax of |p|) - ((half-2) << 7)
    zsub16 = float((half - 2) << 7)
    _S1_CONST = float(255 << 23)  # bits32(2^(1-z)) = this - bits32(2^z)

    nc = bacc.Bacc("TRN2", target_bir_lowering=False, debug=False, num_devices=N_CORES)
    x_ext = nc.dram_tensor("x", [ROWS_PER_CORE, N_COLS], dt.float32, kind="ExternalInput")
    out_ext = nc.dram_tensor("out", [ROWS_PER_CORE, N_COLS], dt.float8e4, kind="ExternalOutput")

    with tile.TileContext(nc) as tc:
        with (
            tc.tile_pool(name="xp", bufs=2) as xp,
            tc.tile_pool(name="yp", bufs=2) as yp,
            tc.tile_pool(name="qp", bufs=2) as qp,
            tc.tile_pool(name="op", bufs=2) as op,
            tc.tile_pool(name="stat", bufs=2) as stat,
        ):
            for s in range(N_SLAB):
                r0 = s * P
                xt = xp.tile([P, N_COLS], dt.float32, tag="x", name=f"x{s}")
                # two half-slab loads (22 KB rows) so compute starts earlier
                H = N_COLS // 2
                nc.sync.dma_start(out=xt[:, 0:H], in_=x_ext[r0 : r0 + P, 0:H])
                nc.sync.dma_start(out=xt[:, H:N_COLS], in_=x_ext[r0 : r0 + P, H:N_COLS])

                yt = yp.tile([P, N_COLS], dt.bfloat16, tag="y", name=f"y{s}")
                rp = stat.tile([P, N_CH], dt.bfloat16, tag="rp", name=f"rp{s}")
                for j in range(N_CH):
                    c0 = j * W
                    nc.vector.tensor_scalar(
                        out=yt[:, c0 : c0 + W], in0=xt[:, c0 : c0 + W],
                        scalar1=C_ADJ, scalar2=None, op0=Alu.mult,
                    )
                yu = yt[:].bitcast(dt.uint16)
                for j in range(N_CH):
                    c0 = j * W
                    # p = y & 0xFF80, in place (sign+exponent only)
                    nc.vector.tensor_scalar(
                        out=yu[:, c0 : c0 + W], in0=yu[:, c0 : c0 + W],
                        scalar1=_EXPSIGN16, scalar2=None, op0=Alu.bitwise_and,
                    )
                    nc.vector.tensor_reduce(
                        out=rp[:, j : j + 1], in_=yt[:, c0 : c0 + W],
                        axis=X, op=Alu.max, apply_absolute_value=True,
                    )

                # --- per-row params ------------------------------------------
                rmax = stat.tile([P, 1], dt.bfloat16, tag="rmax", name=f"rmax{s}")
                nc.vector.tensor_reduce(out=rmax[:], in_=rp[:], axis=X, op=Alu.max)
                # bits16(2^z)
                zb = stat.tile([P, 1], dt.uint16, tag="zb", name=f"zb{s}")
                nc.vector.tensor_scalar(
                    out=zb[:], in0=rmax[:].bitcast(dt.uint16),
                    scalar1=zsub16, scalar2=None, op0=Alu.subtract,
                )
                # s2 = 2^z as f32 (exact bf16->f32 convert via mult by 1)
                s2f = stat.tile([P, 1], dt.float32, tag="s2f", name=f"s2f{s}")
                nc.vector.tensor_scalar(
                    out=s2f[:], in0=zb[:].bitcast(dt.bfloat16),
                    scalar1=1.0, scalar2=None, op0=Alu.mult,
                )
                # s1 = 2^(1-z) via f32 exponent-bit arithmetic
                s1u = stat.tile([P, 1], dt.uint32, tag="s1u", name=f"s1u{s}")
                nc.vector.tensor_scalar(
                    out=s1u[:], in0=s2f[:].bitcast(dt.uint32),
                    scalar1=-1.0, scalar2=_S1_CONST, op0=Alu.mult, op1=Alu.add,
                )
                s1f = s1u[:].bitcast(dt.float32)

                # --- quantize ------------------------------------------------
                qt = qp.tile([P, N_COLS], dt.int16, tag="q", name=f"q{s}")
                ot = op.tile([P, N_COLS], dt.float8e4, tag="o", name=f"o{s}")
                for j in range(N_CH):
                    c0 = j * W
                    # q = int16(p * 2^(1-z)) = +-2^(e-z); e<z truncates to 0
                    nc.vector.tensor_scalar(
                        out=qt[:, c0 : c0 + W], in0=yt[:, c0 : c0 + W],
                        scalar1=s1f, scalar2=None, op0=Alu.mult,
                    )
                    # out = fp8(q * 2^z) = +-2^e
                    nc.scalar.activation(
                        out=ot[:, c0 : c0 + W], in_=qt[:, c0 : c0 + W],
                        func=Act.Copy, bias=0.0, scale=s2f[:],
                    )
                nc.sync.dma_start(out=out_ext[r0 : r0 + P, :], in_=ot[:])

    nc.compile()
    return nc


def kernel(x, n_bits):
    _ensure_path()
    from concourse.bass_utils import run_bass_kernel_spmd

    x = np.ascontiguousarray(np.asarray(x, dtype=np.float32))
    assert x.shape == (N_ROWS, N_COLS), x.shape
    nb = int(np.asarray(n_bits))

    nc = _build(nb)
    in_maps = [
        {"x": x[i * ROWS_PER_CORE : (i + 1) * ROWS_PER_CORE]} for i in range(N_CORES)
    ]
    res = run_bass_kernel_spmd(nc, in_maps, list(range(N_CORES)))
    out8 = np.concatenate([np.asarray(res.results[i]["out"]) for i in range(N_CORES)], axis=0)
    return out8.astype(np.float32)


# revision 4
# speedup vs baseline: 1.6307x; 1.6307x over previous
"""Trainium2 Bass kernel for nn_MinMaxQuantizer (per-channel symmetric log_2 quantizer).

Math (per row c of x[C, D], half = 2**(n_bits-1)):
    maxe    = round(log2(max|x[c, :]|))               # exponent of row max
    z       = maxe - (half - 1)                       # min kept exponent
    e       = round(log2(|x|))                        # per element
    out     = sign(x) * 2^e   if e >= z else 0

Implementation: y = x * c with c = (1 - 2^-9)/sqrt(2) converted to bf16 places
floor_exp(y) = e - 1 exactly (bf16 round-to-nearest carry at the exponent
boundary implements the "mantissa >= sqrt(2)" test).  Then

    p   = y & 0xFF80          (u16 view: sign + exponent, bf16 +-2^(e-1))
    q   = int16(p * 2^(1-z))  = +-2^(e-z), converts to 0 when e < z (|v|<1)
    out = fp8e4m3(q * 2^z)    = +-2^e exactly (power of two, e in [-6, 8])

Row max of |p| is 2^(maxe-1) exactly, so per-row parameters are pure
exponent-bit arithmetic.  fp8 output cuts HBM write traffic 4x; the host
expands to f32 losslessly (every value is a power of two or zero).

Engine split (measured rates): DVE does AND (2x mode), row-max reduce
(1.1 ns/elem, no fast mode exists) and q (2x); ACT does the f32 multiply
(flat 0.97 ns/elem, same cost as any op there) and the fp8 output pass.
This balances both engines at ~80 us/core.

Sharding: rows 4096 -> 8 cores x 512 rows, zero communication.
"""

import sys

import numpy as np

_REPO = "/opt/trn_rl_repo"

N_ROWS = 4096
N_COLS = 11008
N_CORES = 8
ROWS_PER_CORE = N_ROWS // N_CORES  # 512
P = 128
N_SLAB = ROWS_PER_CORE // P  # 4
N_CH = 4
W = N_COLS // N_CH  # 2752
H = N_COLS // 2  # 5504

C_ADJ = float(np.float32((1.0 - 2.0 ** -9) / np.sqrt(2.0)))
_EXPSIGN16 = 0xFF80


def _ensure_path():
    if _REPO not in sys.path:
        sys.path.insert(0, _REPO)


def _build(n_bits: int):
    _ensure_path()
    import concourse.bacc as bacc
    import concourse.mybir as mybir
    import concourse.tile as tile

    dt = mybir.dt
    Alu = mybir.AluOpType
    Act = mybir.ActivationFunctionType
    X = mybir.AxisListType.X

    half = 2 ** (n_bits - 1)
    zsub16 = float((half - 2) << 7)  # bits16(2^z) = bits16(max|p|) - this
    _S1_CONST = float(255 << 23)     # bits32(2^(1-z)) = this - bits32(2^z)

    nc = bacc.Bacc("TRN2", target_bir_lowering=False, debug=False, num_devices=N_CORES)
    x_ext = nc.dram_tensor("x", [ROWS_PER_CORE, N_COLS], dt.float32, kind="ExternalInput")
    out_ext = nc.dram_tensor("out", [ROWS_PER_CORE, N_COLS], dt.float8e4, kind="ExternalOutput")

    with tile.TileContext(nc) as tc:
        with (
            tc.tile_pool(name="xp", bufs=2) as xp,
            tc.tile_pool(name="yp", bufs=2) as yp,
            tc.tile_pool(name="qp", bufs=2) as qp,
            tc.tile_pool(name="op", bufs=2) as op,
            tc.tile_pool(name="stat", bufs=2) as stat,
        ):
            for s in range(N_SLAB):
                r0 = s * P
                # chunk-granular loads keep 16 DMA engines busy; per-chunk
                # tiles give chunk-level dependencies so compute starts after
                # the first 1.4 MB, not the full slab.
                xts = []
                for j in range(N_CH):
                    c0 = j * W
                    xt = xp.tile([P, W], dt.float32, tag=f"x{j}", name=f"x{s}_{j}")
                    nc.sync.dma_start(out=xt[:], in_=x_ext[r0 : r0 + P, c0 : c0 + W])
                    xts.append(xt)

                # y = x * c -> bf16.  ACT takes most multiplies (DVE is the
                # scarcer resource); alternate 3/4 chunks per slab on ACT.
                yt = yp.tile([P, N_COLS], dt.bfloat16, tag="y", name=f"y{s}")
                act_mults = (0, 1, 2) if s % 2 == 0 else (0, 1, 2, 3)
                for j in range(N_CH):
                    c0 = j * W
                    if j in act_mults:
                        nc.scalar.activation(
                            out=yt[:, c0 : c0 + W], in_=xts[j][:],
                            func=Act.Copy, bias=0.0, scale=C_ADJ,
                        )
                    else:
                        nc.vector.tensor_scalar(
                            out=yt[:, c0 : c0 + W], in0=xts[j][:],
                            scalar1=C_ADJ, scalar2=None, op0=Alu.mult,
                        )

                # p = y & 0xFF80 in place (half-slab units, 2x mode), then
                # row-max partials over |p|.
                yu = yt[:].bitcast(dt.uint16)
                rp = stat.tile([P, 2], dt.bfloat16, tag="rp", name=f"rp{s}")
                for hx in range(2):
                    h0 = hx * H
                    nc.vector.tensor_scalar(
                        out=yu[:, h0 : h0 + H], in0=yu[:, h0 : h0 + H],
                        scalar1=_EXPSIGN16, scalar2=None, op0=Alu.bitwise_and,
                    )
                    nc.vector.tensor_reduce(
                        out=rp[:, hx : hx + 1], in_=yt[:, h0 : h0 + H],
                        axis=X, op=Alu.max, apply_absolute_value=True,
                    )

                # --- per-row params ------------------------------------------
                rmax = stat.tile([P, 1], dt.bfloat16, tag="rmax", name=f"rmax{s}")
                nc.vector.tensor_reduce(out=rmax[:], in_=rp[:], axis=X, op=Alu.max)
                zb = stat.tile([P, 1], dt.uint16, tag="zb", name=f"zb{s}")
                nc.vector.tensor_scalar(
                    out=zb[:], in0=rmax[:].bitcast(dt.uint16),
                    scalar1=zsub16, scalar2=None, op0=Alu.subtract,
                )
                s2f = stat.tile([P, 1], dt.float32, tag="s2f", name=f"s2f{s}")
                nc.vector.tensor_scalar(
                    out=s2f[:], in0=zb[:].bitcast(dt.bfloat16),
                    scalar1=1.0, scalar2=None, op0=Alu.mult,
                )
                s1u = stat.tile([P, 1], dt.uint32, tag="s1u", name=f"s1u{s}")
                nc.vector.tensor_scalar(
                    out=s1u[:], in0=s2f[:].bitcast(dt.uint32),
                    scalar1=-1.0, scalar2=_S1_CONST, op0=Alu.mult, op1=Alu.add,
                )
                s1f = s1u[:].bitcast(dt.float32)

                # --- quantize ------------------------------------------------
                qt = qp.tile([P, N_COLS], dt.int16, tag="q", name=f"q{s}")
                for hx in range(2):
                    h0 = hx * H
                    # q = int16(p * 2^(1-z)); e < z truncates to 0 (DVE, 2x)
                    nc.vector.tensor_scalar(
                        out=qt[:, h0 : h0 + H], in0=yt[:, h0 : h0 + H],
                        scalar1=s1f, scalar2=None, op0=Alu.mult,
                    )
                    # out = fp8(q * 2^z) = +-2^e (ACT), then store half-slab
                    ot = op.tile([P, H], dt.float8e4, tag=f"o{hx}", name=f"o{s}_{hx}")
                    nc.scalar.activation(
                        out=ot[:], in_=qt[:, h0 : h0 + H],
                        func=Act.Copy, bias=0.0, scale=s2f[:],
                    )
                    nc.sync.dma_start(
                        out=out_ext[r0 : r0 + P, h0 : h0 + H], in_=ot[:]
                    )

    nc.compile()
    return nc


def kernel(x, n_bits):
    _ensure_path()
    from concourse.bass_utils import run_bass_kernel_spmd

    x = np.ascontiguousarray(np.asarray(x, dtype=np.float32))
    assert x.shape == (N_ROWS, N_COLS), x.shape
    nb = int(np.asarray(n_bits))

    nc = _build(nb)
    in_maps = [
        {"x": x[i * ROWS_PER_CORE : (i + 1) * ROWS_PER_CORE]} for i in range(N_CORES)
    ]
    res = run_bass_kernel_spmd(nc, in_maps, list(range(N_CORES)))
    out8 = np.concatenate([np.asarray(res.results[i]["out"]) for i in range(N_CORES)], axis=0)
    return out8.astype(np.float32)
